# revision 1
# baseline (speedup 1.0000x reference)
"""Trainium2 Bass kernel for nn_AxisNetwork (embedding_lookup + sine MLP).

Math per point (x, y):
    e = lerp(emb0, x) * lerp(emb1, y)          # [256]
    h = sin(30*(e @ w0.T + b0))                # [128]
    h = sin(30*(h @ w1.T + b1))                # [128]
    out = h @ w2.T + b2                        # [3]

Device strategy (pure data parallel over 8 cores, B = N/8 points each):
  * The per-point linear interpolation is replaced by a lookup into a
    U=32x upsampled table (host-precomputed from emb0/emb1, fp16).
    Snapping to the nearest 1/32 sub-cell gives ~5e-4 rel error
    (validated numerically against the f32 reference).
  * Per core: compute int16 row indices from coords on DVE, then
    dma_gather (SWDGE, transpose=True) fetches one 256-wide fp16 row per
    point per axis, landing channel-on-partition: e0T/e1T [128, 2, n].
  * DVE forms e = e0*e1; PE runs the three matmuls with points streaming
    as columns; ACT applies sin(30*z + 30*b) via its scale/bias fold.
  * Output written [128, B/128*3] (point p = n%128, tile n//128);
    host de-interleaves.
"""

import os

import numpy as np

N_FULL = 1 << 20
NCORES = 8
B = int(os.environ.get("KERNEL_B", N_FULL // NCORES))  # points per core
RES = 512
ED = 256
HID = 128
NOUT = 3
W0_FREQ = 30.0

UPS = 32                  # upsample factor for the snap tables
NROWS = (RES - 1) * UPS   # 16352 valid rows
NROWS_PAD = 16384

CHUNK = 4096              # points per gather chunk
STAGE = 1024              # points per compute stage
N_CHUNKS = B // CHUNK
STAGES_PER_CHUNK = CHUNK // STAGE

P = 128

_cache = {}


def _build_nc():
    import concourse.bacc as bacc
    import concourse.bass as bass
    import concourse.mybir as mybir
    import concourse.tile as tile
    from concourse import library_config

    f32 = mybir.dt.float32
    f16 = mybir.dt.float16
    i16 = mybir.dt.int16
    Alu = mybir.AluOpType
    Act = mybir.ActivationFunctionType

    nc = bacc.Bacc("TRN2", target_bir_lowering=False, debug=False,
                   num_devices=NCORES)

    coords_d = nc.dram_tensor("coords", [B, 2], f32, kind="ExternalInput")
    up0_d = nc.dram_tensor("up0", [NROWS_PAD, ED], f16, kind="ExternalInput")
    up1_d = nc.dram_tensor("up1", [NROWS_PAD, ED], f16, kind="ExternalInput")
    w0t_d = nc.dram_tensor("w0t", [2, P, HID], f16, kind="ExternalInput")
    w1t_d = nc.dram_tensor("w1t", [HID, HID], f16, kind="ExternalInput")
    w2t_d = nc.dram_tensor("w2t", [HID, NOUT], f16, kind="ExternalInput")
    b0s_d = nc.dram_tensor("b0s", [P, 1], f32, kind="ExternalInput")
    b1s_d = nc.dram_tensor("b1s", [P, 1], f32, kind="ExternalInput")
    b2t_d = nc.dram_tensor("b2t", [P, (STAGE // P) * NOUT], f32,
                           kind="ExternalInput")
    out_d = nc.dram_tensor("out", [P, (B // P) * NOUT], f32,
                           kind="ExternalOutput")
    # scratch for rearranging indices into the 16-partition-wrapped layout
    xybuf = nc.dram_tensor("xybuf", [2, B], i16)

    FPC = B // P              # free elems per partition per coordinate (1024)
    AFF = 255.5 * UPS         # (0.5c+0.5)*511*UPS == c*AFF + AFF

    with tile.TileContext(nc) as tc:
        with (
            tc.tile_pool(name="const", bufs=1) as cpool,
            tc.tile_pool(name="prep", bufs=1) as prep,
            tc.tile_pool(name="idx", bufs=1) as idxp,
            tc.tile_pool(name="gath", bufs=2) as gath,
            tc.tile_pool(name="act", bufs=2) as actp,
            tc.tile_pool(name="psA", bufs=2, space="PSUM") as psA,
            tc.tile_pool(name="psB", bufs=2, space="PSUM") as psB,
        ):
            nc.gpsimd.load_library(library_config.mlp)

            # ---- constants / weights ----
            w0t = cpool.tile([P, 2, HID], f16)       # [k, c, m]
            nc.sync.dma_start(out=w0t[:], in_=w0t_d[:].rearrange("c k m -> k c m"))
            w1t = cpool.tile([HID, HID], f16)
            nc.sync.dma_start(out=w1t[:], in_=w1t_d[:])
            w2t = cpool.tile([HID, NOUT], f16)
            nc.sync.dma_start(out=w2t[:], in_=w2t_d[:])
            b0s = cpool.tile([P, 1], f32)
            nc.sync.dma_start(out=b0s[:], in_=b0s_d[:])
            b1s = cpool.tile([P, 1], f32)
            nc.sync.dma_start(out=b1s[:], in_=b1s_d[:])
            b2t = cpool.tile([P, (STAGE // P) * NOUT], f32)
            nc.sync.dma_start(out=b2t[:], in_=b2t_d[:])

            out_acc = cpool.tile([P, (B // P) * NOUT], f32)

            # ---- index prep ----
            # coords laid out [p = n%128, f = n//128, axis]
            ctile = prep.tile([P, FPC, 2], f32)
            nc.sync.dma_start(
                out=ctile[:], in_=coords_d[:].rearrange("(f p) a -> p f a", p=P))
            cflat = ctile[:].rearrange("p f a -> p (f a)")
            # clip to [-1, 0.999] (as the reference does), then affine to
            # upsampled-row coordinates; round via f32->int16 convert.
            cl = prep.tile([P, FPC * 2], f32)
            nc.vector.tensor_scalar(out=cl[:], in0=cflat, scalar1=0.999,
                                    scalar2=-1.0, op0=Alu.min, op1=Alu.max)
            av = prep.tile([P, FPC * 2], f32)
            nc.vector.tensor_scalar(out=av[:], in0=cl[:], scalar1=AFF,
                                    scalar2=AFF, op0=Alu.mult, op1=Alu.add)
            idx16 = prep.tile([P, FPC * 2], i16)
            nc.vector.tensor_copy(out=idx16[:], in_=av[:])

            # scatter x/y indices to DRAM in point order
            for a in range(2):
                nc.sync.dma_start(
                    out=xybuf[a].rearrange("(f p) -> p f", p=P),
                    in_=idx16[:].rearrange("p (f a) -> a p f", a=2)[a])
            # reload wrapped-by-16, replicated into all 8 partition groups
            idxs = []
            for a in range(2):
                t = idxp.tile([P, B // 16], i16, tag=f"idxs{a}")
                for g in range(8):
                    nc.sync.dma_start(
                        out=t[16 * g:16 * (g + 1), :],
                        in_=xybuf[a].rearrange("(f q) -> q f", q=16))
                idxs.append(t)

            # ---- main pipeline ----
            for k in range(N_CHUNKS):
                e0 = gath.tile([P, 2, CHUNK], f16, tag="e0")
                e1 = gath.tile([P, 2, CHUNK], f16, tag="e1")
                ncol = CHUNK // 16
                nc.gpsimd.dma_gather(
                    e0[:], up0_d[:], idxs[0][:, k * ncol:(k + 1) * ncol],
                    num_idxs=CHUNK, num_idxs_reg=CHUNK, elem_size=ED,
                    transpose=True, single_packet=False)
                nc.gpsimd.dma_gather(
                    e1[:], up1_d[:], idxs[1][:, k * ncol:(k + 1) * ncol],
                    num_idxs=CHUNK, num_idxs_reg=CHUNK, elem_size=ED,
                    transpose=True, single_packet=False)
                ee = gath.tile([P, 2, CHUNK], f16, tag="ee")
                nc.vector.tensor_tensor(
                    out=ee[:].rearrange("p c n -> p (c n)"),
                    in0=e0[:].rearrange("p c n -> p (c n)"),
                    in1=e1[:].rearrange("p c n -> p (c n)"),
                    op=Alu.mult)

                for si in range(STAGES_PER_CHUNK):
                    s = k * STAGES_PER_CHUNK + si
                    lo = si * STAGE
                    # layer 0: z0[h, n] = sum_d w0[h, d] e[d, n]
                    z0 = psA.tile([P, STAGE], f32, tag="z0", space="PSUM")
                    for half in range(STAGE // 512):
                        cs = lo + half * 512
                        for c in range(2):
                            nc.tensor.matmul(
                                z0[:, half * 512:(half + 1) * 512],
                                w0t[:, c, :],
                                ee[:, c, cs:cs + 512],
                                start=(c == 0), stop=(c == 1))
                    h0 = actp.tile([P, STAGE], f16, tag="h0")
                    nc.scalar.activation(out=h0[:], in_=z0[:], func=Act.Sin,
                                         bias=b0s[:], scale=W0_FREQ)
                    # layer 1 (w1t is pre-scaled by 30 on the host; ACT's Sin
                    # spline only covers [-pi, pi], so wrap 30*z1+30*b1 back
                    # into range by one period first — sin is 2pi-periodic)
                    z1 = psB.tile([P, STAGE], f32, tag="zb")
                    for half in range(STAGE // 512):
                        nc.tensor.matmul(
                            z1[:, half * 512:(half + 1) * 512],
                            w1t[:],
                            h0[:, half * 512:(half + 1) * 512],
                            start=True, stop=True)
                    t1 = actp.tile([P, STAGE], f32, tag="t1")
                    nc.vector.add_range_wrap(out=t1[:], in_=z1[:], shift=b1s[:],
                                             bound=float(np.pi),
                                             period=float(2 * np.pi))
                    h1 = actp.tile([P, STAGE], f16, tag="h1")
                    nc.scalar.activation(out=h1[:], in_=t1[:], func=Act.Sin)
                    # layer 2 (points become the stationary M dim)
                    o2 = psB.tile([P, (STAGE // P) * NOUT], f32, tag="zb")
                    for t in range(STAGE // P):
                        nc.tensor.matmul(
                            o2[:, t * NOUT:(t + 1) * NOUT],
                            h1[:, t * P:(t + 1) * P],
                            w2t[:],
                            start=True, stop=True)
                    nc.vector.scalar_tensor_tensor(
                        out=out_acc[:, s * (STAGE // P) * NOUT:
                                    (s + 1) * (STAGE // P) * NOUT],
                        in0=o2[:], scalar=1.0, in1=b2t[:],
                        op0=Alu.mult, op1=Alu.add)

            nc.sync.dma_start(out=out_d[:], in_=out_acc[:])

    nc.compile()
    return nc


def _host_prep(inputs):
    coords = np.ascontiguousarray(inputs["coords"], dtype=np.float32)
    emb0 = np.asarray(inputs["emb0"], dtype=np.float32)
    emb1 = np.asarray(inputs["emb1"], dtype=np.float32)
    w0 = np.asarray(inputs["w0"], dtype=np.float32)
    b0 = np.asarray(inputs["b0"], dtype=np.float32)
    w1 = np.asarray(inputs["w1"], dtype=np.float32)
    b1 = np.asarray(inputs["b1"], dtype=np.float32)
    w2 = np.asarray(inputs["w2"], dtype=np.float32)
    b2 = np.asarray(inputs["b2"], dtype=np.float32)

    def upsample(emb):
        i = np.arange(RES - 1)
        w = (np.arange(UPS, dtype=np.float64) / UPS).astype(np.float32)
        t = (1.0 - w)[None, :, None] * emb[i][:, None, :] \
            + w[None, :, None] * emb[i + 1][:, None, :]
        t = t.reshape(NROWS, ED)
        pad = np.zeros((NROWS_PAD - NROWS, ED), np.float32)
        return np.concatenate([t, pad], 0).astype(np.float16)

    up0 = upsample(emb0)
    up1 = upsample(emb1)
    w0t = np.ascontiguousarray(
        w0.T.reshape(2, P, HID)).astype(np.float16)        # [c, k, m]
    w1t = np.ascontiguousarray(w1.T * W0_FREQ).astype(np.float16)  # [k, m], pre-scaled
    w2t = np.ascontiguousarray(w2.T).astype(np.float16)    # [k, 3]
    b0s = (W0_FREQ * b0).reshape(P, 1).astype(np.float32)
    b1s = (W0_FREQ * b1).reshape(P, 1).astype(np.float32)
    b2t = np.tile(b2, STAGE // P).reshape(1, -1).repeat(P, 0).astype(np.float32)

    shared = dict(up0=up0, up1=up1, w0t=w0t, w1t=w1t, w2t=w2t,
                  b0s=b0s, b1s=b1s, b2t=b2t)
    in_maps = []
    for c in range(NCORES):
        shard = np.ascontiguousarray(coords[c * B:(c + 1) * B])
        in_maps.append(dict(coords=shard, **shared))
    return in_maps


last_results = None


def kernel(**inputs):
    global last_results
    from concourse.bass_utils import run_bass_kernel_spmd
    import os

    if "nc" not in _cache:
        _cache["nc"] = _build_nc()
    nc = _cache["nc"]

    in_maps = _host_prep(inputs)
    trace = bool(int(os.environ.get("KERNEL_TRACE", "0")))
    res = run_bass_kernel_spmd(nc, in_maps, core_ids=list(range(NCORES)),
                               trace=trace)
    last_results = res

    outs = []
    for c in range(NCORES):
        dev = res.results[c]["out"]                  # [128, (B/128)*3]
        dev = dev.reshape(P, B // P, NOUT).transpose(1, 0, 2).reshape(B, NOUT)
        outs.append(dev)
    return np.ascontiguousarray(
        np.concatenate(outs, 0).astype(np.float32))



# revision 10
# speedup vs baseline: 1.3665x; 1.3665x over previous
"""Trainium2 Bass kernel for nn_AxisNetwork (embedding_lookup + sine MLP).

Math per point (x, y):
    e = lerp(emb0, x) * lerp(emb1, y)          # [256]
    h = sin(30*(e @ w0.T + b0))                # [128]
    h = sin(30*(h @ w1.T + b1))                # [128]
    out = h @ w2.T + b2                        # [3]

Device strategy (pure data parallel over 8 cores, B = N/8 points each):
  * Lookup via a U=32x upsampled snap table (host-precomputed, fp16),
    ~5e-4 rel error.
  * dma_gather WITHOUT transpose: each point fetches one contiguous
    512B row per axis, landing point-on-partition: e_raw [128, T, 256]
    (point n = t*128 + p).  The transpose=True gather mode shatters
    rows into 2-byte partition writes and is ~20x slower on HW.
  * DVE forms ee = e0*e1 in raw layout (f16, fast mode).
  * PE transposes ee per 128-point tile into PSUM (f16, packed 8 tiles
    per bank); one DVE/ACT copy per 512-pt group moves it to SBUF.
  * MLP: l0 (w0 stationary, 2 K-chunks), ACT sin; l1 (w1*30 stationary),
    DVE range-wrap + ACT sin; l2 per-tile (h1 stationary, w2 moving)
    accumulating onto a bias-prefilled PSUM bank.
  * Output written [128, B/128*3] (point p = n%128, tile n//128);
    host de-interleaves.
  * Stages are software-pipelined: transposes of stage S overlap MLP
    phases of stages S-1/S-2/S-3 so PE never waits on ACT/DVE.
"""

import os

import numpy as np

N_FULL = 1 << 20
NCORES = 8
B = int(os.environ.get("KERNEL_B", N_FULL // NCORES))  # points per core
RES = 512
ED = 256
HID = 128
NOUT = 3
W0_FREQ = 30.0

UPS = 32                  # upsample factor for the snap tables
NROWS = (RES - 1) * UPS   # 16352 valid rows
NROWS_PAD = 16384

CHUNK = 4096              # points per gather chunk
STAGE = 1024              # points per MLP stage
GROUP = 512               # points per transpose/copy group
N_CHUNKS = B // CHUNK
STAGES_PER_CHUNK = CHUNK // STAGE
N_STAGES = N_CHUNKS * STAGES_PER_CHUNK

P = 128
TPC = CHUNK // P          # gather tiles per chunk (32)

_cache = {}


def _build_nc():
    import concourse.bacc as bacc
    import concourse.bass as bass
    import concourse.mybir as mybir
    import concourse.tile as tile
    from concourse import library_config

    f32 = mybir.dt.float32
    f16 = mybir.dt.float16
    i16 = mybir.dt.int16
    Alu = mybir.AluOpType
    Act = mybir.ActivationFunctionType

    nc = bacc.Bacc("TRN2", target_bir_lowering=False, debug=False,
                   num_devices=NCORES)

    coords_d = nc.dram_tensor("coords", [B, 2], f32, kind="ExternalInput")
    up0_d = nc.dram_tensor("up0", [NROWS_PAD, ED], f16, kind="ExternalInput")
    up1_d = nc.dram_tensor("up1", [NROWS_PAD, ED], f16, kind="ExternalInput")
    w0t_d = nc.dram_tensor("w0t", [2, P, HID], f16, kind="ExternalInput")
    w1t_d = nc.dram_tensor("w1t", [HID, HID], f16, kind="ExternalInput")
    w2t_d = nc.dram_tensor("w2t", [HID, NOUT], f16, kind="ExternalInput")
    b0s_d = nc.dram_tensor("b0s", [P, 1], f32, kind="ExternalInput")
    b1s_d = nc.dram_tensor("b1s", [P, 1], f32, kind="ExternalInput")
    b2r_d = nc.dram_tensor("b2r", [P, (CHUNK // P) * NOUT], f32,
                           kind="ExternalInput")
    id_d = nc.dram_tensor("ident", [P, P], f16, kind="ExternalInput")
    out_d = nc.dram_tensor("out", [P, (B // P) * NOUT], f32,
                           kind="ExternalOutput")
    DBG = bool(int(os.environ.get("KERNEL_DBG", "0")))
    if DBG:
        dbg_ee_d = nc.dram_tensor("dbg_ee", [P, 2 * GROUP], f16,
                                  kind="ExternalOutput")   # eeT S=0 g=0
        dbg_h0_d = nc.dram_tensor("dbg_h0", [P, STAGE], f16,
                                  kind="ExternalOutput")   # h0 stage 0
        dbg_h1_d = nc.dram_tensor("dbg_h1", [P, STAGE], f16,
                                  kind="ExternalOutput")   # h1 stage 0
    # scratch for rearranging indices into the 16-partition-wrapped layout
    xybuf = nc.dram_tensor("xybuf", [2, B], i16)

    FPC = B // P              # free elems per partition per coordinate
    AFF = 255.5 * UPS         # (0.5c+0.5)*511*UPS == c*AFF + AFF
    OC = (CHUNK // P) * NOUT  # psO cols per chunk (96)

    with tile.TileContext(nc) as tc:
        with (
            tc.tile_pool(name="const", bufs=1) as cpool,
            tc.tile_pool(name="prep", bufs=1) as prep,
            tc.tile_pool(name="idx", bufs=1) as idxp,
            tc.tile_pool(name="gath", bufs=2) as gath,
            tc.tile_pool(name="eep", bufs=2) as eep,
            tc.tile_pool(name="eet", bufs=4) as eetp,
            tc.tile_pool(name="act", bufs=2) as actp,
            tc.tile_pool(name="psT", bufs=2, space="PSUM") as psT,
            tc.tile_pool(name="psZ0", bufs=1, space="PSUM") as psZ0,
            tc.tile_pool(name="psZ1", bufs=1, space="PSUM") as psZ1,
            tc.tile_pool(name="psO", bufs=2, space="PSUM") as psO,
        ):
            nc.gpsimd.load_library(library_config.mlp)

            # ---- constants / weights ----
            w0t = cpool.tile([P, 2, HID], f16)       # [k, c, m]
            nc.sync.dma_start(out=w0t[:], in_=w0t_d[:].rearrange("c k m -> k c m"))
            w1t = cpool.tile([HID, HID], f16)
            nc.sync.dma_start(out=w1t[:], in_=w1t_d[:])
            w2t = cpool.tile([HID, NOUT], f16)
            nc.sync.dma_start(out=w2t[:], in_=w2t_d[:])
            b0s = cpool.tile([P, 1], f32)
            nc.sync.dma_start(out=b0s[:], in_=b0s_d[:])
            b1s = cpool.tile([P, 1], f32)
            nc.sync.dma_start(out=b1s[:], in_=b1s_d[:])
            b2r = cpool.tile([P, OC], f32)
            nc.sync.dma_start(out=b2r[:], in_=b2r_d[:])
            ident = cpool.tile([P, P], f16)
            nc.sync.dma_start(out=ident[:], in_=id_d[:])

            out_acc = cpool.tile([P, (B // P) * NOUT], f32)

            # ---- index prep ----
            ctile = prep.tile([P, FPC, 2], f32)
            nc.sync.dma_start(
                out=ctile[:], in_=coords_d[:].rearrange("(f p) a -> p f a", p=P))
            cflat = ctile[:].rearrange("p f a -> p (f a)")
            cl = prep.tile([P, FPC * 2], f32)
            nc.vector.tensor_scalar(out=cl[:], in0=cflat, scalar1=0.999,
                                    scalar2=-1.0, op0=Alu.min, op1=Alu.max)
            av = prep.tile([P, FPC * 2], f32)
            nc.vector.tensor_scalar(out=av[:], in0=cl[:], scalar1=AFF,
                                    scalar2=AFF, op0=Alu.mult, op1=Alu.add)
            idx16 = prep.tile([P, FPC * 2], i16)
            nc.vector.tensor_copy(out=idx16[:], in_=av[:])

            for a in range(2):
                nc.sync.dma_start(
                    out=xybuf[a].rearrange("(f p) -> p f", p=P),
                    in_=idx16[:].rearrange("p (f a) -> a p f", a=2)[a])
            idxs = []
            for a in range(2):
                t = idxp.tile([P, B // 16], i16, tag=f"idxs{a}")
                for g in range(8):
                    nc.sync.dma_start(
                        out=t[16 * g:16 * (g + 1), :],
                        in_=xybuf[a].rearrange("(f q) -> q f", q=16))
                idxs.append(t)

            # ---- pipelined main loop ----
            # state carried between pipeline phases, keyed by stage index
            live = {}

            def emit_chunk_head(k):
                e0 = gath.tile([P, TPC, ED], f16, tag="e0")
                e1 = gath.tile([P, TPC, ED], f16, tag="e1")
                ncol = CHUNK // 16
                nc.gpsimd.dma_gather(
                    e0[:], up0_d[:], idxs[0][:, k * ncol:(k + 1) * ncol],
                    num_idxs=CHUNK, num_idxs_reg=CHUNK, elem_size=ED,
                    transpose=False, single_packet=False)
                nc.gpsimd.dma_gather(
                    e1[:], up1_d[:], idxs[1][:, k * ncol:(k + 1) * ncol],
                    num_idxs=CHUNK, num_idxs_reg=CHUNK, elem_size=ED,
                    transpose=False, single_packet=False)
                ee = eep.tile([P, TPC, ED], f16, tag="ee")
                nc.vector.tensor_tensor(
                    out=ee[:].rearrange("p t d -> p (t d)"),
                    in0=e0[:].rearrange("p t d -> p (t d)"),
                    in1=e1[:].rearrange("p t d -> p (t d)"),
                    op=Alu.mult)
                po = psO.tile([P, OC], f32, tag="po")
                return ee, po

            def emit_transposes(s, ee):
                # two 512-pt groups; each: 8 transposes -> 1 bank -> 1 copy
                eets = []
                for g2 in range(2):
                    tb = psT.tile([P, 2 * GROUP], f16, tag="tb")
                    base = (s % STAGES_PER_CHUNK) * (STAGE // P) \
                        + g2 * (GROUP // P)
                    for t in range(GROUP // P):
                        for h in range(2):
                            nc.tensor.transpose(
                                tb[:, h * GROUP + t * P:
                                   h * GROUP + (t + 1) * P],
                                ee[:, base + t, h * P:(h + 1) * P],
                                ident[:])
                    eet = eetp.tile([P, 2, GROUP], f16, tag="eet")
                    if g2 == 0:
                        nc.vector.tensor_copy(
                            out=eet[:].rearrange("p c n -> p (c n)"),
                            in_=tb[:])
                    else:
                        nc.scalar.copy(
                            out=eet[:].rearrange("p c n -> p (c n)"),
                            in_=tb[:])
                    if DBG and s == 0 and g2 == 0:
                        nc.sync.dma_start(
                            out=dbg_ee_d[:],
                            in_=eet[:].rearrange("p c n -> p (c n)"))
                    eets.append(eet)
                return eets

            def emit_l0(st):
                z0 = psZ0.tile([P, STAGE], f32, tag="z0")
                for g2 in range(2):
                    for c in range(2):
                        nc.tensor.matmul(
                            z0[:, g2 * GROUP:(g2 + 1) * GROUP],
                            w0t[:, c, :],
                            st["eets"][g2][:, c, :],
                            start=(c == 0), stop=(c == 1))
                h0 = actp.tile([P, STAGE], f16, tag="h0")
                nc.scalar.activation(out=h0[:], in_=z0[:], func=Act.Sin,
                                     bias=b0s[:], scale=W0_FREQ)
                if DBG and st["s"] == 0:
                    nc.sync.dma_start(out=dbg_h0_d[:], in_=h0[:])
                st["h0"] = h0

            def emit_l1(st):
                z1 = psZ1.tile([P, STAGE], f32, tag="z1")
                for g2 in range(2):
                    nc.tensor.matmul(
                        z1[:, g2 * GROUP:(g2 + 1) * GROUP],
                        w1t[:],
                        st["h0"][:, g2 * GROUP:(g2 + 1) * GROUP],
                        start=True, stop=True)
                t1 = actp.tile([P, STAGE], f32, tag="t1")
                nc.vector.add_range_wrap(out=t1[:], in_=z1[:], shift=b1s[:],
                                         bound=float(np.pi),
                                         period=float(2 * np.pi))
                h1 = actp.tile([P, STAGE], f16, tag="h1")
                nc.scalar.activation(out=h1[:], in_=t1[:], func=Act.Sin)
                if DBG and st["s"] == 0:
                    nc.sync.dma_start(out=dbg_h1_d[:], in_=h1[:])
                st["h1"] = h1

            def emit_l2(st):
                po = st["po"]
                si = st["s"] % STAGES_PER_CHUNK
                for t in range(STAGE // P):
                    nc.tensor.matmul(
                        po[:, (si * (STAGE // P) + t) * NOUT:
                           (si * (STAGE // P) + t + 1) * NOUT],
                        st["h1"][:, t * P:(t + 1) * P],
                        w2t[:],
                        start=True, stop=True)
                st["done"] = True

            def emit_chunk_tail(k, po):
                nc.vector.scalar_tensor_tensor(
                    out=out_acc[:, k * OC:(k + 1) * OC],
                    in0=po[:], scalar=1.0, in1=b2r[:],
                    op0=Alu.mult, op1=Alu.add)

            ee_cur = po_cur = None
            chunk_done = {}
            for S in range(N_STAGES + 3):
                if S < N_STAGES:
                    k, si = divmod(S, STAGES_PER_CHUNK)
                    if si == 0:
                        ee_cur, po_cur = emit_chunk_head(k)
                        chunk_done[k] = 0
                    eets = emit_transposes(S, ee_cur)
                    live[S] = {"s": S, "k": k, "eets": eets, "po": po_cur}
                if S - 1 >= 0 and S - 1 < N_STAGES:
                    emit_l0(live[S - 1])
                if S - 2 >= 0 and S - 2 < N_STAGES:
                    emit_l1(live[S - 2])
                if S - 3 >= 0 and S - 3 < N_STAGES:
                    st = live.pop(S - 3)
                    emit_l2(st)
                    kk = st["k"]
                    chunk_done[kk] += 1
                    if chunk_done[kk] == STAGES_PER_CHUNK:
                        emit_chunk_tail(kk, st["po"])

            nc.sync.dma_start(out=out_d[:], in_=out_acc[:])

    nc.compile()
    return nc


def _host_prep(inputs):
    coords = np.ascontiguousarray(inputs["coords"], dtype=np.float32)
    emb0 = np.asarray(inputs["emb0"], dtype=np.float32)
    emb1 = np.asarray(inputs["emb1"], dtype=np.float32)
    w0 = np.asarray(inputs["w0"], dtype=np.float32)
    b0 = np.asarray(inputs["b0"], dtype=np.float32)
    w1 = np.asarray(inputs["w1"], dtype=np.float32)
    b1 = np.asarray(inputs["b1"], dtype=np.float32)
    w2 = np.asarray(inputs["w2"], dtype=np.float32)
    b2 = np.asarray(inputs["b2"], dtype=np.float32)

    def upsample(emb):
        i = np.arange(RES - 1)
        w = (np.arange(UPS, dtype=np.float64) / UPS).astype(np.float32)
        t = (1.0 - w)[None, :, None] * emb[i][:, None, :] \
            + w[None, :, None] * emb[i + 1][:, None, :]
        t = t.reshape(NROWS, ED)
        pad = np.zeros((NROWS_PAD - NROWS, ED), np.float32)
        return np.concatenate([t, pad], 0).astype(np.float16)

    up0 = upsample(emb0)
    up1 = upsample(emb1)
    w0t = np.ascontiguousarray(
        w0.T.reshape(2, P, HID)).astype(np.float16)        # [c, k, m]
    w1t = np.ascontiguousarray(w1.T * W0_FREQ).astype(np.float16)
    w2t = np.ascontiguousarray(w2.T).astype(np.float16)    # [k, 3]
    b0s = (W0_FREQ * b0).reshape(P, 1).astype(np.float32)
    b1s = (W0_FREQ * b1).reshape(P, 1).astype(np.float32)
    b2r = np.tile(b2, CHUNK // P).reshape(1, -1).repeat(P, 0).astype(np.float32)
    ident = np.eye(P, dtype=np.float16)

    shared = dict(up0=up0, up1=up1, w0t=w0t, w1t=w1t, w2t=w2t,
                  b0s=b0s, b1s=b1s, b2r=b2r, ident=ident)
    in_maps = []
    for c in range(NCORES):
        shard = np.ascontiguousarray(coords[c * B:(c + 1) * B])
        in_maps.append(dict(coords=shard, **shared))
    return in_maps


last_results = None


def kernel(**inputs):
    global last_results
    from concourse.bass_utils import run_bass_kernel_spmd
    import os

    if "nc" not in _cache:
        _cache["nc"] = _build_nc()
    nc = _cache["nc"]

    in_maps = _host_prep(inputs)
    trace = bool(int(os.environ.get("KERNEL_TRACE", "0")))
    res = run_bass_kernel_spmd(nc, in_maps, core_ids=list(range(NCORES)),
                               trace=trace)
    last_results = res

    outs = []
    for c in range(NCORES):
        dev = res.results[c]["out"]                  # [128, (B/128)*3]
        dev = dev.reshape(P, B // P, NOUT).transpose(1, 0, 2).reshape(B, NOUT)
        outs.append(dev)
    return np.ascontiguousarray(
        np.concatenate(outs, 0).astype(np.float32))


# revision 14
# speedup vs baseline: 3.3467x; 2.4491x over previous
"""Trainium2 Bass kernel for nn_AxisNetwork (embedding_lookup + sine MLP).

Math per point (x, y):
    e = lerp(emb0, x) * lerp(emb1, y)          # [256]
    h = sin(30*(e @ w0.T + b0))                # [128]
    h = sin(30*(h @ w1.T + b1))                # [128]
    out = h @ w2.T + b2                        # [3]

Device strategy (pure data parallel over 8 cores, B = N/8 points each):
  * Lookup via a U=32x upsampled snap table (host-precomputed, fp16),
    ~5e-4 rel error.
  * dma_gather WITHOUT transpose: each point fetches one contiguous
    512B row per axis, landing point-on-partition: e_raw [128, T, 256]
    (point n = t*128 + p).  The transpose=True gather mode shatters
    rows into 2-byte partition writes and is ~20x slower on HW.
  * DVE forms ee = e0*e1 in raw layout (f16, fast mode).
  * PE transposes ee per 128-point tile into PSUM (f16, packed 8 tiles
    per bank); one DVE/ACT copy per 512-pt group moves it to SBUF.
  * MLP: l0 (w0 stationary, 2 K-chunks), ACT sin; l1 (w1*30 stationary),
    DVE range-wrap + ACT sin; l2 per-tile (h1 stationary, w2 moving)
    accumulating onto a bias-prefilled PSUM bank.
  * Output written [128, B/128*3] (point p = n%128, tile n//128);
    host de-interleaves.
  * Stages are software-pipelined: transposes of stage S overlap MLP
    phases of stages S-1/S-2/S-3 so PE never waits on ACT/DVE.
"""

import os

import numpy as np

N_FULL = 1 << 20
NCORES = 8
B = int(os.environ.get("KERNEL_B", N_FULL // NCORES))  # points per core
RES = 512
ED = 256
HID = 128
NOUT = 3
W0_FREQ = 30.0

UPS = 32                  # upsample factor for the snap tables
NROWS = (RES - 1) * UPS   # 16352 valid rows
NROWS_PAD = 16384

CHUNK = 4096              # points per gather chunk
STAGE = 1024              # points per MLP stage
GROUP = 512               # points per transpose/copy group
N_CHUNKS = B // CHUNK
STAGES_PER_CHUNK = CHUNK // STAGE
N_STAGES = N_CHUNKS * STAGES_PER_CHUNK

P = 128
TPC = CHUNK // P          # gather tiles per chunk (32)

_cache = {}


def _build_nc():
    import concourse.bacc as bacc
    import concourse.bass as bass
    import concourse.mybir as mybir
    import concourse.tile as tile
    from concourse import library_config

    f32 = mybir.dt.float32
    f16 = mybir.dt.float16
    i16 = mybir.dt.int16
    Alu = mybir.AluOpType
    Act = mybir.ActivationFunctionType

    nc = bacc.Bacc("TRN2", target_bir_lowering=False, debug=False,
                   num_devices=NCORES)

    coords_d = nc.dram_tensor("coords", [B, 2], f32, kind="ExternalInput")
    up0_d = nc.dram_tensor("up0", [NROWS_PAD, ED], f16, kind="ExternalInput")
    up1_d = nc.dram_tensor("up1", [NROWS_PAD, ED], f16, kind="ExternalInput")
    w0t_d = nc.dram_tensor("w0t", [2, P, HID], f16, kind="ExternalInput")
    w1t_d = nc.dram_tensor("w1t", [HID, HID], f16, kind="ExternalInput")
    w2t_d = nc.dram_tensor("w2t", [HID, NOUT], f16, kind="ExternalInput")
    b0s_d = nc.dram_tensor("b0s", [P, 1], f32, kind="ExternalInput")
    b1s_d = nc.dram_tensor("b1s", [P, 1], f32, kind="ExternalInput")
    b2r_d = nc.dram_tensor("b2r", [P, (CHUNK // P) * NOUT], f32,
                           kind="ExternalInput")
    id_d = nc.dram_tensor("ident", [P, P], f16, kind="ExternalInput")
    out_d = nc.dram_tensor("out", [P, (B // P) * NOUT], f32,
                           kind="ExternalOutput")
    DBG = bool(int(os.environ.get("KERNEL_DBG", "0")))
    if DBG:
        dbg_ee_d = nc.dram_tensor("dbg_ee", [P, 2 * GROUP], f16,
                                  kind="ExternalOutput")   # eeT S=0 g=0
        dbg_h0_d = nc.dram_tensor("dbg_h0", [P, STAGE], f16,
                                  kind="ExternalOutput")   # h0 stage 0
        dbg_h1_d = nc.dram_tensor("dbg_h1", [P, STAGE], f16,
                                  kind="ExternalOutput")   # h1 stage 0
    # scratch for rearranging indices into the 16-partition-wrapped layout
    xybuf = nc.dram_tensor("xybuf", [2, B], i16)

    FPC = B // P              # free elems per partition per coordinate
    AFF = 255.5 * UPS         # (0.5c+0.5)*511*UPS == c*AFF + AFF
    OC = (CHUNK // P) * NOUT  # psO cols per chunk (96)

    with tile.TileContext(nc) as tc:
        with (
            tc.tile_pool(name="const", bufs=1) as cpool,
            tc.tile_pool(name="prep", bufs=1) as prep,
            tc.tile_pool(name="idx", bufs=1) as idxp,
            tc.tile_pool(name="gath", bufs=2) as gath,
            tc.tile_pool(name="eep", bufs=2) as eep,
            tc.tile_pool(name="eet", bufs=4) as eetp,
            tc.tile_pool(name="act", bufs=2) as actp,
            tc.tile_pool(name="psT", bufs=2, space="PSUM") as psT,
            tc.tile_pool(name="psZ0", bufs=1, space="PSUM") as psZ0,
            tc.tile_pool(name="psZ1", bufs=1, space="PSUM") as psZ1,
            tc.tile_pool(name="psO", bufs=2, space="PSUM") as psO,
        ):
            nc.gpsimd.load_library(library_config.mlp)

            # ---- constants / weights ----
            w0t = cpool.tile([P, 2, HID], f16)       # [k, c, m]
            nc.sync.dma_start(out=w0t[:], in_=w0t_d[:].rearrange("c k m -> k c m"))
            w1t = cpool.tile([HID, HID], f16)
            nc.sync.dma_start(out=w1t[:], in_=w1t_d[:])
            w2t = cpool.tile([HID, NOUT], f16)
            nc.sync.dma_start(out=w2t[:], in_=w2t_d[:])
            b0s = cpool.tile([P, 1], f32)
            nc.sync.dma_start(out=b0s[:], in_=b0s_d[:])
            b1s = cpool.tile([P, 1], f32)
            nc.sync.dma_start(out=b1s[:], in_=b1s_d[:])
            b2r = cpool.tile([P, OC], f32)
            nc.sync.dma_start(out=b2r[:], in_=b2r_d[:])
            ident = cpool.tile([P, P], f16)
            nc.sync.dma_start(out=ident[:], in_=id_d[:])

            out_acc = cpool.tile([P, (B // P) * NOUT], f32)

            # ---- index prep ----
            ctile = prep.tile([P, FPC, 2], f32)
            nc.sync.dma_start(
                out=ctile[:], in_=coords_d[:].rearrange("(f p) a -> p f a", p=P))
            cflat = ctile[:].rearrange("p f a -> p (f a)")
            cl = prep.tile([P, FPC * 2], f32)
            nc.vector.tensor_scalar(out=cl[:], in0=cflat, scalar1=0.999,
                                    scalar2=-1.0, op0=Alu.min, op1=Alu.max)
            av = prep.tile([P, FPC * 2], f32)
            nc.vector.tensor_scalar(out=av[:], in0=cl[:], scalar1=AFF,
                                    scalar2=AFF, op0=Alu.mult, op1=Alu.add)
            idx16 = prep.tile([P, FPC * 2], i16)
            nc.vector.tensor_copy(out=idx16[:], in_=av[:])

            for a in range(2):
                nc.sync.dma_start(
                    out=xybuf[a].rearrange("(f p) -> p f", p=P),
                    in_=idx16[:].rearrange("p (f a) -> a p f", a=2)[a])
            idxs = []
            for a in range(2):
                t = idxp.tile([P, B // 16], i16, tag=f"idxs{a}")
                for g in range(8):
                    nc.sync.dma_start(
                        out=t[16 * g:16 * (g + 1), :],
                        in_=xybuf[a].rearrange("(f q) -> q f", q=16))
                idxs.append(t)

            # ---- pipelined main loop ----
            # state carried between pipeline phases, keyed by stage index
            live = {}

            def emit_chunk_head(k):
                e0 = gath.tile([P, TPC, ED], f16, tag="e0")
                e1 = gath.tile([P, TPC, ED], f16, tag="e1")
                ncol = CHUNK // 16
                nc.gpsimd.dma_gather(
                    e0[:], up0_d[:], idxs[0][:, k * ncol:(k + 1) * ncol],
                    num_idxs=CHUNK, num_idxs_reg=CHUNK, elem_size=ED,
                    transpose=False, single_packet=False)
                nc.gpsimd.dma_gather(
                    e1[:], up1_d[:], idxs[1][:, k * ncol:(k + 1) * ncol],
                    num_idxs=CHUNK, num_idxs_reg=CHUNK, elem_size=ED,
                    transpose=False, single_packet=False)
                ee = eep.tile([P, TPC, ED], f16, tag="ee")
                nc.vector.tensor_tensor(
                    out=ee[:].rearrange("p t d -> p (t d)"),
                    in0=e0[:].rearrange("p t d -> p (t d)"),
                    in1=e1[:].rearrange("p t d -> p (t d)"),
                    op=Alu.mult)
                po = psO.tile([P, OC], f32, tag="po")
                return ee, po

            def emit_transposes(s, ee):
                # two 512-pt groups; each: 8 transposes -> 1 bank -> 1 copy
                eets = []
                for g2 in range(2):
                    tb = psT.tile([P, 2 * GROUP], f16, tag="tb")
                    base = (s % STAGES_PER_CHUNK) * (STAGE // P) \
                        + g2 * (GROUP // P)
                    for t in range(GROUP // P):
                        for h in range(2):
                            nc.tensor.transpose(
                                tb[:, h * GROUP + t * P:
                                   h * GROUP + (t + 1) * P],
                                ee[:, base + t, h * P:(h + 1) * P],
                                ident[:])
                    eet = eetp.tile([P, 2, GROUP], f16, tag="eet")
                    if g2 == 0:
                        nc.vector.tensor_copy(
                            out=eet[:].rearrange("p c n -> p (c n)"),
                            in_=tb[:])
                    else:
                        nc.scalar.copy(
                            out=eet[:].rearrange("p c n -> p (c n)"),
                            in_=tb[:])
                    if DBG and s == 0 and g2 == 0:
                        nc.sync.dma_start(
                            out=dbg_ee_d[:],
                            in_=eet[:].rearrange("p c n -> p (c n)"))
                    eets.append(eet)
                return eets

            def emit_l0(st):
                z0 = psZ0.tile([P, STAGE], f32, tag="z0")
                for g2 in range(2):
                    for c in range(2):
                        nc.tensor.matmul(
                            z0[:, g2 * GROUP:(g2 + 1) * GROUP],
                            w0t[:, c, :],
                            st["eets"][g2][:, c, :],
                            start=(c == 0), stop=(c == 1))
                h0 = actp.tile([P, STAGE], f16, tag="h0")
                nc.scalar.activation(out=h0[:], in_=z0[:], func=Act.Sin,
                                     bias=b0s[:], scale=W0_FREQ)
                if DBG and st["s"] == 0:
                    nc.sync.dma_start(out=dbg_h0_d[:], in_=h0[:])
                st["h0"] = h0

            def emit_l1(st):
                z1 = psZ1.tile([P, STAGE], f32, tag="z1")
                for g2 in range(2):
                    nc.tensor.matmul(
                        z1[:, g2 * GROUP:(g2 + 1) * GROUP],
                        w1t[:],
                        st["h0"][:, g2 * GROUP:(g2 + 1) * GROUP],
                        start=True, stop=True)
                t1 = actp.tile([P, STAGE], f32, tag="t1")
                nc.vector.add_range_wrap(out=t1[:], in_=z1[:], shift=b1s[:],
                                         bound=float(np.pi),
                                         period=float(2 * np.pi))
                h1 = actp.tile([P, STAGE], f16, tag="h1")
                nc.scalar.activation(out=h1[:], in_=t1[:], func=Act.Sin)
                if DBG and st["s"] == 0:
                    nc.sync.dma_start(out=dbg_h1_d[:], in_=h1[:])
                st["h1"] = h1

            def emit_l2(st):
                po = st["po"]
                si = st["s"] % STAGES_PER_CHUNK
                for t in range(STAGE // P):
                    nc.tensor.matmul(
                        po[:, (si * (STAGE // P) + t) * NOUT:
                           (si * (STAGE // P) + t + 1) * NOUT],
                        st["h1"][:, t * P:(t + 1) * P],
                        w2t[:],
                        start=True, stop=True)
                st["done"] = True

            def emit_chunk_tail(k, po):
                nc.vector.scalar_tensor_tensor(
                    out=out_acc[:, k * OC:(k + 1) * OC],
                    in0=po[:], scalar=1.0, in1=b2r[:],
                    op0=Alu.mult, op1=Alu.add)

            ee_cur = po_cur = None
            chunk_done = {}
            for S in range(N_STAGES + 3):
                if S < N_STAGES:
                    k, si = divmod(S, STAGES_PER_CHUNK)
                    if si == 0:
                        ee_cur, po_cur = emit_chunk_head(k)
                        chunk_done[k] = 0
                    eets = emit_transposes(S, ee_cur)
                    live[S] = {"s": S, "k": k, "eets": eets, "po": po_cur}
                if S - 1 >= 0 and S - 1 < N_STAGES:
                    emit_l0(live[S - 1])
                if S - 2 >= 0 and S - 2 < N_STAGES:
                    emit_l1(live[S - 2])
                if S - 3 >= 0 and S - 3 < N_STAGES:
                    st = live.pop(S - 3)
                    emit_l2(st)
                    kk = st["k"]
                    chunk_done[kk] += 1
                    if chunk_done[kk] == STAGES_PER_CHUNK:
                        emit_chunk_tail(kk, st["po"])

            nc.sync.dma_start(out=out_d[:], in_=out_acc[:])

    nc.compile()
    return nc


def _host_prep(inputs):
    coords = np.ascontiguousarray(inputs["coords"], dtype=np.float32)
    emb0 = np.asarray(inputs["emb0"], dtype=np.float32)
    emb1 = np.asarray(inputs["emb1"], dtype=np.float32)
    w0 = np.asarray(inputs["w0"], dtype=np.float32)
    b0 = np.asarray(inputs["b0"], dtype=np.float32)
    w1 = np.asarray(inputs["w1"], dtype=np.float32)
    b1 = np.asarray(inputs["b1"], dtype=np.float32)
    w2 = np.asarray(inputs["w2"], dtype=np.float32)
    b2 = np.asarray(inputs["b2"], dtype=np.float32)

    def upsample(emb):
        i = np.arange(RES - 1)
        w = (np.arange(UPS, dtype=np.float64) / UPS).astype(np.float32)
        t = (1.0 - w)[None, :, None] * emb[i][:, None, :] \
            + w[None, :, None] * emb[i + 1][:, None, :]
        t = t.reshape(NROWS, ED)
        pad = np.zeros((NROWS_PAD - NROWS, ED), np.float32)
        return np.concatenate([t, pad], 0).astype(np.float16)

    up0 = upsample(emb0)
    up1 = upsample(emb1)
    w0t = np.ascontiguousarray(
        w0.T.reshape(2, P, HID)).astype(np.float16)        # [c, k, m]
    w1t = np.ascontiguousarray(w1.T * W0_FREQ).astype(np.float16)
    w2t = np.ascontiguousarray(w2.T).astype(np.float16)    # [k, 3]
    b0s = (W0_FREQ * b0).reshape(P, 1).astype(np.float32)
    b1s = (W0_FREQ * b1).reshape(P, 1).astype(np.float32)
    b2r = np.tile(b2, CHUNK // P).reshape(1, -1).repeat(P, 0).astype(np.float32)
    ident = np.eye(P, dtype=np.float16)

    shared = dict(up0=up0, up1=up1, w0t=w0t, w1t=w1t, w2t=w2t,
                  b0s=b0s, b1s=b1s, b2r=b2r, ident=ident)
    in_maps = []
    for c in range(NCORES):
        shard = np.ascontiguousarray(coords[c * B:(c + 1) * B])
        in_maps.append(dict(coords=shard, **shared))
    return in_maps


last_results = None


def kernel(**inputs):
    global last_results
    from concourse.bass_utils import run_bass_kernel_spmd
    import os

    if "nc" not in _cache:
        _cache["nc"] = _build_nc()
    nc = _cache["nc"]

    in_maps = _host_prep(inputs)
    trace = bool(int(os.environ.get("KERNEL_TRACE", "0")))
    res = run_bass_kernel_spmd(nc, in_maps, core_ids=list(range(NCORES)),
                               trace=trace)
    last_results = res

    outs = []
    for c in range(NCORES):
        dev = res.results[c]["out"]                  # [128, (B/128)*3]
        dev = dev.reshape(P, B // P, NOUT).transpose(1, 0, 2).reshape(B, NOUT)
        outs.append(dev)
    return np.ascontiguousarray(
        np.concatenate(outs, 0).astype(np.float32))
